# revision 27
# baseline (speedup 1.0000x reference)
"""Trainium2 Bass kernel for the prototype-bank scatter-mean EMA update
(nn_Bank): class-sharded sorted-segment reduction across 8 NeuronCores.

Host (index/layout work only; all FP reduction arithmetic is on device):
  * argsort labels; assign each class to one core (greedy token balance,
    <=128 classes/core); concatenate each core's class segments, zero-
    padding every class to a multiple of GTOK tokens so each GTOK-token
    "block" is single-class; features shipped bf16, token-major.
  * per-core metadata: a one-hot row per block over the core's local
    class ids (zero row for pad blocks), per-block valid-token counts,
    and the prototype rows for the core's classes.

Device, per core (T tiles; tile = 128 blocks = 128*GTOK tokens):
  1. Stream feature tiles [128, GTOK*64] bf16 HBM->SBUF (sync queue);
     one-hot tiles [128, 128] bf16 stream on the scalar-engine queue.
  2. Partial block sums via contiguous pairwise halving adds, split
     between DVE (2 levels -> [128, 512]) and GpSimd (1 level ->
     [128, 1024]) to balance engine throughput.
  3. PE: psum_a/b[cls, 512] += oh^T @ partials (PSUM-accumulated over
     all tiles; the matmul absorbs the remaining reduction levels), and
     psum_c[cls, 1] += oh^T @ blk_cnt -> per-class counts.
  4. Blend: fold psum partials -> sums; means = sums/max(cnt,1);
     out = proto + s*(means-proto), s = present*(0.1 + 0.9*use_new).
  5. DMA out [128, 64]; host scatters per-core rows back to [1000, 64].

No collective: every class is fully owned by one core.
"""

import numpy as np

import concourse.bacc as bacc
import concourse.bass as bass
import concourse.mybir as mybir
from concourse import bass_utils

C = 1000
D = 64
E = D + 1            # feature dims + count indicator (metadata only)
P = 128
GTOK = 32            # tokens per block (class padding granularity)
LAM = 0.9
WARMUP_STEP = 1000
N_CORES = 8
NB = 16              # feature tile buffers
NR = 8               # rhs buffers
NO = 8               # one-hot tile buffers
FW = D * GTOK        # free elems per feature tile partition
RW = 8 * D           # rhs cols handed to PE (8 partial sums x 64 dims)


OHB = 8              # one-hot batch size (tiles per build instruction)


def tile_on_dve(j: int) -> bool:
    # hybrid: DVE tiles get a 2-level tree (PE streams 512 cols), Pool
    # tiles 1 level (PE streams 2x512); ~41:21 balances DVE/Pool/PE
    return j % 3 != 2


def build_nc(T: int, step_gt_warmup: bool):
    f32 = mybir.dt.float32
    bf16 = mybir.dt.bfloat16
    fp8 = mybir.dt.float8e4

    dcount = [0] * (T + 1)  # dcount[j+1] = #DVE tiles among 0..j
    pcount = [0] * (T + 1)
    for j in range(T):
        dcount[j + 1] = dcount[j] + (1 if tile_on_dve(j) else 0)
        pcount[j + 1] = pcount[j] + (0 if tile_on_dve(j) else 1)

    nc = bacc.Bacc("TRN2", target_bir_lowering=False, debug=False,
                   num_devices=N_CORES)

    feat = nc.dram_tensor("feature", [T * P, FW], bf16, kind="ExternalInput")
    oh_rows = nc.dram_tensor("oh_rows", [T * P, P], bf16, kind="ExternalInput")
    meta2 = nc.dram_tensor("blk_cnt", [P, T], bf16, kind="ExternalInput")
    proto = nc.dram_tensor("prototype", [P, D], f32, kind="ExternalInput")
    out = nc.dram_tensor("out", [P, D], f32, kind="ExternalOutput")

    ftiles = [nc.alloc_sbuf_tensor(f"ftile{b}", [P, FW], bf16) for b in range(NB)]
    meta2_sb = nc.alloc_sbuf_tensor("meta2_sb", [P, T], bf16)
    proto_sb = nc.alloc_sbuf_tensor("proto_sb", [P, D], f32)
    rhs = [nc.alloc_sbuf_tensor(f"rhs{i}", [P, FW // 2], bf16) for i in range(NR)]
    oh = [nc.alloc_sbuf_tensor(f"oh{i}", [P, P], bf16) for i in range(NO)]
    cnt = nc.alloc_sbuf_tensor("cnt", [P, 1], f32)
    rcp = nc.alloc_sbuf_tensor("rcp", [P, 1], f32)
    pres = nc.alloc_sbuf_tensor("pres", [P, 1], f32)
    znorm = nc.alloc_sbuf_tensor("znorm", [P, 1], f32)
    svec = nc.alloc_sbuf_tensor("svec", [P, 1], f32)
    means = nc.alloc_sbuf_tensor("means", [P, D], f32)
    dtile = nc.alloc_sbuf_tensor("dtile", [P, D], f32)
    otile = nc.alloc_sbuf_tensor("otile", [P, D], f32)

    psum_a = nc.alloc_psum_tensor("psum_a", [P, RW], f32)
    psum_b = nc.alloc_psum_tensor("psum_b", [P, RW], f32)
    psum_c = nc.alloc_psum_tensor("psum_c", [P, 1], f32)
    sums = nc.alloc_sbuf_tensor("sums", [P, D], f32)

    lsems = [nc.alloc_semaphore(f"lsem{b}") for b in range(NB)]
    rsem_d = nc.alloc_semaphore("rsem_d")  # DVE reduces done
    rsem_p = nc.alloc_semaphore("rsem_p")  # Pool reduces done
    osems = [nc.alloc_semaphore(f"osem{i}") for i in range(NO)]
    msem = nc.alloc_semaphore("msem")      # matmuls done (1 per tile)
    psem = nc.alloc_semaphore("psem")      # preamble loads (3 x 16)
    bsem = nc.alloc_semaphore("bsem")      # blend done
    fsem = nc.alloc_semaphore("fsem")      # out store done
    vch = nc.alloc_semaphore("vch")        # blend chain

    def pstride(t):
        return t.ap().ap[0][0]

    def feat_tile_ap(j):
        return bass.AP(feat, j * P * FW, [[FW, P], [1, FW]])

    def ftile_red_ap(b):
        t = ftiles[b]
        return bass.AP(t, 0, [[pstride(t), P], [GTOK, E], [1, GTOK]])

    def col(t, c, w=1):
        return bass.AP(t, c, [[pstride(t), P], [1, w]])

    def wait_reduced(eng, j):
        """Wait until tile j's rhs is ready."""
        if tile_on_dve(j):
            eng.wait_ge(rsem_d, dcount[j + 1])
        else:
            eng.wait_ge(rsem_p, pcount[j + 1])

    def tree_reduce(eng, j, rsem, levels):
        """1 or 2 halving adds: ftile[b] [P, FW] -> rhs[j%NR] ([P, FW/2]
        in cols 0:FW/2, or [P, RW] in cols 0:RW for levels=2)."""
        b = j % NB
        t = ftiles[b]
        r = rhs[j % NR]
        eng.wait_ge(lsems[b], 16 * (j // NB + 1))
        h = FW // 2
        if levels == 2:
            eng.tensor_tensor(
                bass.AP(t, 0, [[pstride(t), P], [1, h]]),
                bass.AP(t, 0, [[pstride(t), P], [1, h]]),
                bass.AP(t, h, [[pstride(t), P], [1, h]]),
                mybir.AluOpType.add,
            )
            if j >= NR:
                eng.wait_ge(msem, j - NR + 1)
            eng.tensor_tensor(
                bass.AP(r, 0, [[pstride(r), P], [1, RW]]),
                bass.AP(t, 0, [[pstride(t), P], [1, RW]]),
                bass.AP(t, RW, [[pstride(t), P], [1, RW]]),
                mybir.AluOpType.add,
            ).then_inc(rsem, 1)
        else:
            if j >= NR:
                eng.wait_ge(msem, j - NR + 1)
            eng.tensor_tensor(
                bass.AP(r, 0, [[pstride(r), P], [1, h]]),
                bass.AP(t, 0, [[pstride(t), P], [1, h]]),
                bass.AP(t, h, [[pstride(t), P], [1, h]]),
                mybir.AluOpType.add,
            ).then_inc(rsem, 1)

    with nc.allow_low_precision("bf16 block sums; exact count col"), \
            nc.Block() as block:

        @block.scalar
        def _(scalar):
            scalar.dma_start(proto_sb.ap(), proto.ap()).then_inc(psem, 16)
            scalar.dma_start(meta2_sb.ap(), meta2.ap()).then_inc(psem, 16)
            for j in range(T):
                if j >= NO:
                    scalar.wait_ge(msem, j - NO + 1)
                scalar.dma_start(
                    oh[j % NO].ap(),
                    bass.AP(oh_rows, j * P * P, [[P, P], [1, P]]),
                ).then_inc(osems[j % NO], 16)

        @block.sync
        def _(sync):
            for j in range(T):
                b = j % NB
                if j >= NB:
                    wait_reduced(sync, j - NB)
                sync.dma_start(ftiles[b].ap(), feat_tile_ap(j)).then_inc(lsems[b], 16)
            sync.wait_ge(bsem, 1)
            sync.dma_start(out.ap(), otile.ap()).then_inc(fsem, 16)
            sync.wait_ge(fsem, 16)

        @block.gpsimd
        def _(gpsimd):
            for j in range(T):
                if not tile_on_dve(j):
                    tree_reduce(gpsimd, j, rsem_p, 1)

        @block.vector
        def _(vector):
            vector.wait_ge(psem, 32)
            for j in range(T):
                if tile_on_dve(j):
                    tree_reduce(vector, j, rsem_d, 2)

            # ---- blend ----
            vector.wait_ge(msem, T)
            vc = [0]

            def chain(ins):
                ins.then_inc(vch, 1)
                vc[0] += 1
                vector.wait_ge(vch, vc[0])

            chain(vector.tensor_reduce(
                sums.ap(),
                bass.AP(psum_a, 0, [[pstride(psum_a), P], [1, D], [D, 8]]),
                axis=mybir.AxisListType.X, op=mybir.AluOpType.add,
            ))
            chain(vector.tensor_reduce(
                dtile.ap(),
                bass.AP(psum_b, 0, [[pstride(psum_b), P], [1, D], [D, 8]]),
                axis=mybir.AxisListType.X, op=mybir.AluOpType.add,
            ))
            chain(vector.tensor_tensor(sums.ap(), sums.ap(), dtile.ap(),
                                       mybir.AluOpType.add))
            chain(vector.tensor_copy(cnt.ap(), psum_c.ap()))
            chain(vector.tensor_scalar_max(rcp.ap(), cnt.ap(), 1.0))
            chain(vector.reciprocal(rcp.ap(), rcp.ap()))
            chain(vector.tensor_scalar(pres.ap(), cnt.ap(), 0.5, None,
                                       mybir.AluOpType.is_gt))
            if step_gt_warmup:
                chain(vector.tensor_reduce(
                    znorm.ap(), proto_sb.ap(),
                    axis=mybir.AxisListType.X, op=mybir.AluOpType.max,
                    apply_absolute_value=True,
                ))
                chain(vector.tensor_scalar(svec.ap(), znorm.ap(), 0.0, None,
                                           mybir.AluOpType.is_equal))
            else:
                chain(vector.memset(svec.ap(), 1.0))
            # svec = pres * (0.1 + 0.9*use_new)
            chain(vector.tensor_scalar(svec.ap(), svec.ap(), LAM, 1.0 - LAM,
                                       mybir.AluOpType.mult,
                                       mybir.AluOpType.add))
            chain(vector.tensor_tensor(svec.ap(), svec.ap(), pres.ap(),
                                       mybir.AluOpType.mult))
            chain(vector.tensor_scalar_mul(means.ap(), sums.ap(), col(rcp, 0)))
            chain(vector.tensor_tensor(dtile.ap(), means.ap(), proto_sb.ap(),
                                       mybir.AluOpType.subtract))
            vector.scalar_tensor_tensor(
                otile.ap(), dtile.ap(), col(svec, 0), proto_sb.ap(),
                mybir.AluOpType.mult, mybir.AluOpType.add,
            ).then_inc(bsem, 1)

        @block.tensor
        def _(tensor):
            pool_tiles = [j for j in range(T) if not tile_on_dve(j)]
            tensor.wait_ge(psem, 32)
            for j in range(T):
                wait_reduced(tensor, j)
                tensor.wait_ge(osems[j % NO], 16 * (j // NO + 1))
                oh_ap = oh[j % NO].ap()
                r = rhs[j % NR]
                tensor.matmul(
                    psum_a.ap(), oh_ap,
                    bass.AP(r, 0, [[pstride(r), P], [1, RW]]),
                    start=(j == 0), stop=(j == T - 1),
                )
                if not tile_on_dve(j):
                    tensor.matmul(
                        psum_b.ap(), oh_ap,
                        bass.AP(r, RW, [[pstride(r), P], [1, RW]]),
                        start=(j == pool_tiles[0]), stop=(j == pool_tiles[-1]),
                    )
                tensor.matmul(
                    psum_c.ap(), oh_ap,
                    bass.AP(meta2_sb, j, [[pstride(meta2_sb), P], [1, 1]]),
                    start=(j == 0), stop=(j == T - 1),
                ).then_inc(msem, 1)

    nc.compile()
    return nc


def shard_inputs(feature, label, prototype):
    """Returns (in_maps, cls_lists, T)."""
    import ml_dtypes
    bf16 = ml_dtypes.bfloat16
    fp8 = ml_dtypes.float8_e4m3

    counts = np.bincount(label, minlength=C)

    # greedy: biggest class -> least-loaded core (cap 128 classes/core)
    order_cls = np.argsort(-counts, kind="stable")
    core_load = np.zeros(N_CORES, dtype=np.int64)
    core_ncls = np.zeros(N_CORES, dtype=np.int64)
    cls_lists = [[] for _ in range(N_CORES)]
    nblk = (counts + GTOK - 1) // GTOK  # blocks per class
    for c in order_cls:
        k = min((k for k in range(N_CORES) if core_ncls[k] < P),
                key=lambda k: core_load[k])
        cls_lists[k].append(c)
        core_load[k] += nblk[c]
        core_ncls[k] += 1

    T = int(max(1, -(-core_load.max() // P)))
    cap_blk = T * P
    cap_tok = cap_blk * GTOK

    sort_order = np.argsort(label, kind="stable")
    starts = np.zeros(C + 1, dtype=np.int64)
    np.cumsum(counts, out=starts[1:])

    feat_bf = np.ascontiguousarray(feature, dtype=np.float32).astype(bf16)

    src_all = np.full(N_CORES * cap_tok, -1, dtype=np.int64)
    metas = []
    for k in range(N_CORES):
        base = k * cap_tok
        pos = 0
        mcls = np.full(cap_blk, -1, dtype=np.int64)
        mcnt = np.zeros(cap_blk, dtype=np.float32)
        blk = 0
        for li, c in enumerate(cls_lists[k]):
            ncv = int(counts[c])
            if ncv:
                src_all[base + pos: base + pos + ncv] = \
                    sort_order[starts[c]: starts[c] + ncv]
            nb = int(nblk[c])
            if nb:
                mcls[blk: blk + nb] = li
                mcnt[blk: blk + nb] = GTOK
                mcnt[blk + nb - 1] = ncv - (nb - 1) * GTOK
            pos += nb * GTOK
            blk += nb
        metas.append((mcls, mcnt))

    nblk_tot = N_CORES * cap_blk
    valid = src_all >= 0
    tok = np.zeros((nblk_tot * GTOK, D), dtype=bf16)
    tok[valid] = feat_bf[src_all[valid]]
    arr = tok.reshape(N_CORES, T * P, FW)

    proto32 = np.ascontiguousarray(prototype, dtype=np.float32)
    in_maps = []
    for k in range(N_CORES):
        cl = np.asarray(cls_lists[k], dtype=np.int64)
        pk = np.zeros((P, D), dtype=np.float32)
        pk[: len(cl)] = proto32[cl]
        # one-hot row per block (zero row for pad blocks)
        mcls = metas[k][0]
        ohh = np.zeros((cap_blk, P), dtype=bf16)
        real = mcls >= 0
        ohh[np.flatnonzero(real), mcls[real]] = 1
        # block b=(tile j, partition p) -> blk_cnt[p, j]
        meta2_k = np.ascontiguousarray(
            metas[k][1].reshape(T, P).T.astype(bf16))
        in_maps.append({
            "feature": np.ascontiguousarray(arr[k]),
            "oh_rows": ohh,
            "blk_cnt": meta2_k,
            "prototype": pk,
        })
    return in_maps, cls_lists, T


_NC_CACHE = {}


def run(inputs: dict, trace: bool = False):
    feature = np.asarray(inputs["feature"])
    label = np.asarray(inputs["label"], dtype=np.int64)
    prototype = np.asarray(inputs["prototype"])
    step = int(np.asarray(inputs["step"]))

    in_maps, cls_lists, T = shard_inputs(feature, label, prototype)
    key = (T, step > WARMUP_STEP)
    if key not in _NC_CACHE:
        _NC_CACHE[key] = build_nc(T, step > WARMUP_STEP)
    nc = _NC_CACHE[key]
    res = bass_utils.run_bass_kernel_spmd(
        nc, in_maps, core_ids=list(range(N_CORES)), trace=trace,
    )
    out = np.ascontiguousarray(prototype, dtype=np.float32).copy()
    for k in range(N_CORES):
        cl = np.asarray(cls_lists[k], dtype=np.int64)
        ok = np.asarray(res.results[k]["out"], dtype=np.float32)
        out[cl] = ok[: len(cl)]
    return out, res


def kernel(**inputs) -> np.ndarray:
    out, _ = run(inputs, trace=False)
    return out
